# revision 1
# baseline (speedup 1.0000x reference)
"""Trainium2 Bass kernel for a single-layer transformer encoder.

Model: B=2, N=2048, D=1024, H=16, DFF=4096 (pre-computed QKV attention +
residual/LN + GELU FFN + residual/LN).

Sharding (zero-collective): 2 batches x 4-way sequence split. Core c owns
the 512 query tokens q=c%4 of batch b=c//4 and recomputes K/V for its whole
batch locally (~1.37x compute redundancy, but no collectives at all).

Device layout is feature-major ("transposed"): activations are stored as
[feature, token] so every projection's weight matrix is the natural
stationary (lhsT) operand and activations stream as the moving operand.
Softmax runs on transposed scores PT[j, i] = exp(scale * k_j . q_i); the
denominators come for free from a ones-column appended to V (out partition
64 of the attention-output accumulation), so no cross-partition reduction
is ever needed. LayerNorm reductions over the feature (partition) dim are
done with ones-vector matmuls on the PE; per-token mean/rstd are broadcast
back across partitions with rank-1 (k=1) fp32 matmuls (exact).

Precision: matmul operands are bf16 (input rounding washes out through the
LayerNorms; ~7e-4 rel err end to end), while the residual spine (x_own,
z1, xln1 residual copy, z2) and all LN statistics stay fp32.

Performance structure: per-core DMA sustains only ~116 GB/s, so the 16 MB
of FFN weights are streamed on the gpsimd (SWDGE) queues starting at
attention time — the attention phase is the only compute window with no
DMA of its own. Attention exp runs on ScalarE over [128, 1024] double-bank
PSUM tiles (amortizes the +352-cycle ACTIVATE overhead); all copies and
scale/bias applies run on VectorE so ACT never switches LUT tables.
SBUF slots are aliased across phases via same-tag pool rotation
(QT/kt/vp/xow slots are reused for z1/xln1f/z2 once their readers finish).
"""

import os
import sys

for _p in ("/opt/trn_rl_repo", "/root/.axon_site", "/root/.axon_site/_ro/trn_rl_repo"):
    if os.path.isdir(_p) and _p not in sys.path:
        sys.path.append(_p)

import numpy as np

import concourse.bacc as bacc
import concourse.mybir as mybir
import concourse.tile as tile
from concourse.bass_utils import run_bass_kernel_spmd

P = 128
B, NSEQ, D, H, DFF = 2, 2048, 1024, 16, 4096
DH = D // H                     # 64
NT = 512                        # query tokens per core
DM = D // P                     # 8 feature chunks
JC = NSEQ // P                  # 16 key-token chunks
TC = NSEQ // 512                # 4 512-token chunks
FC = DFF // P                   # 32 FFN feature chunks
HPAIRS = H // 2                 # 8
SCALE = DH ** -0.5
EPS = 1e-5

F32 = mybir.dt.float32
BF16 = mybir.dt.bfloat16
AF = mybir.ActivationFunctionType

_NC_CACHE = None


def _rearr(ap):
    """DRAM [D_like, T] -> [p, chunk, T] view with chunk-major features."""
    return ap.rearrange("(c p) t -> p c t", p=P)


def _build_nc(reps=1, phases=("qkv", "attn", "proj", "ffn")):
    nc = bacc.Bacc("TRN2", target_bir_lowering=False, debug=False)

    xT = nc.dram_tensor("xT", [D, NSEQ], BF16, kind="ExternalInput")
    x_own = nc.dram_tensor("x_own", [D, NT], F32, kind="ExternalInput")
    # weights arrive pre-tiled: [out_chunk, partition, in_chunk, out_cols]
    w_q = nc.dram_tensor("w_q", [DM, P, DM, P], BF16, kind="ExternalInput")
    w_k = nc.dram_tensor("w_k", [DM, P, DM, P], BF16, kind="ExternalInput")
    w_v = nc.dram_tensor("w_v", [2, P, DM, 512], BF16, kind="ExternalInput")
    w_out = nc.dram_tensor("w_out", [DM, P, DM, P], BF16, kind="ExternalInput")
    w1 = nc.dram_tensor("w1", [DFF // 512, P, DM, 512], BF16,
                        kind="ExternalInput")
    w2 = nc.dram_tensor("w2", [DM, P, FC, P], BF16, kind="ExternalInput")
    b1 = nc.dram_tensor("b1", [DFF], F32, kind="ExternalInput")
    b2 = nc.dram_tensor("b2", [D], F32, kind="ExternalInput")
    ln1_w = nc.dram_tensor("ln1_w", [D], F32, kind="ExternalInput")
    ln1_b = nc.dram_tensor("ln1_b", [D], F32, kind="ExternalInput")
    ln2_w = nc.dram_tensor("ln2_w", [D], F32, kind="ExternalInput")
    ln2_b = nc.dram_tensor("ln2_b", [D], F32, kind="ExternalInput")
    yT = nc.dram_tensor("yT", [D, NT], F32, kind="ExternalOutput")

    with tile.TileContext(nc) as tc, \
         nc.allow_low_precision(reason="bf16 matmul operands; fp32 spine"):
        for _ in range(reps):
            _emit(nc, tc, xT, x_own, w_q, w_k, w_v, w_out, w1, w2, b1, b2,
                  ln1_w, ln1_b, ln2_w, ln2_b, yT, phases=phases)
    nc.compile()
    return nc


def _emit(nc, tc, xT_d, xown_d, w_q, w_k, w_v, w_out, w1, w2, b1, b2,
          ln1_w, ln1_b, ln2_w, ln2_b, yT_d,
          phases=("qkv", "attn", "proj", "ffn")):
    # ---------------- whole-kernel pools ----------------
    with tc.tile_pool(name="const", bufs=1) as pc, \
         tc.tile_pool(name="pers", bufs=1) as pers, \
         tc.tile_pool(name="scratch", bufs=3) as sq_pool, \
         tc.tile_pool(name="vecs", bufs=4) as vec_pool, \
         tc.tile_pool(name="psacc", bufs=2, space="PSUM") as psacc, \
         tc.tile_pool(name="pspt", bufs=2, space="PSUM") as pspt, \
         tc.tile_pool(name="psout", bufs=2, space="PSUM") as psout:

        # ---------------- constants ----------------
        ones_f32 = pc.tile([P, 2 * P], F32)
        nc.vector.memset(ones_f32[:], 1.0)
        ones_col = pc.tile([P, 1], BF16)          # lhsT for partition-sums
        nc.vector.tensor_copy(ones_col[:], ones_f32[:, 0:1])
        ones_row = pc.tile([1, P], F32)           # lhsT for exact broadcasts
        nc.vector.tensor_copy(ones_row[:], ones_f32[0:1, 0:P])
        eps_sb = pc.tile([1, 1], F32)
        nc.vector.memset(eps_sb[:], EPS)
        b1_sb = pc.tile([P, FC], F32)
        nc.sync.dma_start(b1_sb[:], b1.ap().rearrange("(c p) -> p c", p=P))
        b2_sb = pc.tile([P, DM], F32)
        nc.sync.dma_start(b2_sb[:], b2.ap().rearrange("(c p) -> p c", p=P))
        lnw1_sb = pc.tile([P, DM], F32)
        nc.sync.dma_start(lnw1_sb[:], ln1_w.ap().rearrange("(c p) -> p c", p=P))
        lnb1_sb = pc.tile([P, DM], F32)
        nc.sync.dma_start(lnb1_sb[:], ln1_b.ap().rearrange("(c p) -> p c", p=P))
        lnw2_sb = pc.tile([P, DM], F32)
        nc.sync.dma_start(lnw2_sb[:], ln2_w.ap().rearrange("(c p) -> p c", p=P))
        lnb2_sb = pc.tile([P, DM], F32)
        nc.sync.dma_start(lnb2_sb[:], ln2_b.ap().rearrange("(c p) -> p c", p=P))

        # persistent activations (z2 reuses xow's slot via tag rotation)
        QT = pers.tile([P, DM, NT], BF16)
        outT = pers.tile([P, DM, NT], BF16)
        xow = pers.tile([P, DM, NT], F32, tag="tc")  # own-token x (residual 1)
        xln1 = pers.tile([P, DM, NT], BF16)     # LN1 out (ffn matmul operand)

        nc.sync.dma_start(xow[:], _rearr(xown_d.ap()))

        def ln_apply(z_tile, writes):
            """LayerNorm over features of z_tile [P, DM, NT] (fp32).
            writes(k, src_f32_ap) stores chunk k."""
            # bf16 shadow for the PE stat reductions (errors average out)
            s1 = psacc.tile([1, NT], F32, tag="acc")
            s2 = psacc.tile([1, NT], F32, tag="acc")
            for k in range(DM):
                zb = sq_pool.tile([P, NT], BF16, tag="sq")
                nc.vector.tensor_copy(zb[:], z_tile[:, k, :])
                nc.tensor.matmul(s1[:], ones_col[:], zb[:],
                                 start=(k == 0), stop=(k == DM - 1))
                sq = sq_pool.tile([P, NT], BF16, tag="sq")
                nc.vector.tensor_mul(sq[:], zb[:], zb[:])
                nc.tensor.matmul(s2[:], ones_col[:], sq[:],
                                 start=(k == 0), stop=(k == DM - 1))
            mu = vec_pool.tile([1, NT], F32, tag="v")
            nc.vector.tensor_scalar_mul(mu[:], s1[:], 1.0 / D)
            var = vec_pool.tile([1, NT], F32, tag="v")
            nc.vector.tensor_scalar_mul(var[:], s2[:], 1.0 / D)
            musq = vec_pool.tile([1, NT], F32, tag="v")
            nc.vector.tensor_mul(musq[:], mu[:], mu[:])
            nc.vector.tensor_sub(var[:], var[:], musq[:])
            nc.scalar.activation(var[:], var[:], AF.Sqrt, bias=eps_sb[:])
            rec = vec_pool.tile([1, NT], F32, tag="v")
            nc.vector.reciprocal(rec[:], var[:])
            murf = vec_pool.tile([1, NT], F32, tag="v")
            nc.vector.tensor_mul(murf[:], mu[:], rec[:])
            R = psacc.tile([P, NT], F32, tag="acc")
            nc.tensor.matmul(R[:], ones_row[:], rec[:], start=True, stop=True)
            MR = psacc.tile([P, NT], F32, tag="acc")
            nc.tensor.matmul(MR[:], ones_row[:], murf[:], start=True, stop=True)
            for k in range(DM):
                t = sq_pool.tile([P, NT], F32, tag="sq")
                nc.vector.tensor_mul(t[:], z_tile[:, k, :], R[:])
                nc.vector.tensor_sub(t[:], t[:], MR[:])
                writes(k, t)

        with tc.tile_pool(name="ktp", bufs=1) as kt_pool, \
             tc.tile_pool(name="vpp", bufs=1) as vp_pool, \
             tc.tile_pool(name="w1p", bufs=4) as w1_pool:

            with tc.tile_pool(name="xpool", bufs=1) as px:
                xT = px.tile([P, DM, NSEQ], BF16)
                xTs = _rearr(xT_d.ap())
                for k in range(DM):
                    nc.sync.dma_start(xT[:, k, :], xTs[:, k, :])

                # -------- projections: Q, V, K (dense PE block) ----------
                with tc.tile_pool(name="wq", bufs=2) as wq_pool:
                    for qf in range(DM):
                        wq = wq_pool.tile([P, DM, P], BF16)
                        nc.sync.dma_start(wq[:], w_q.ap()[qf])
                        acc = psacc.tile([P, NT], F32, tag="acc")
                        for k in range(DM):
                            nc.tensor.matmul(acc[:], wq[:, k, :],
                                             xT[:, k, 0:NT],
                                             start=(k == 0), stop=(k == DM - 1))
                        nc.vector.tensor_copy(QT[:, qf, :], acc[:])

                vp = vp_pool.tile([P, JC, H * 65], BF16, tag="vp")
                vp_h = vp.rearrange("p j (h e) -> p j h e", e=65)
                nc.vector.tensor_copy(
                    vp_h[:, :, :, 64:65],
                    ones_f32.rearrange("p (a b c) -> p a b c", b=H, c=1))
                with tc.tile_pool(name="wv", bufs=2) as wv_pool:
                    for dvc in range(2):  # 512 v-features = 8 heads at a time
                        wv = wv_pool.tile([P, DM, 512], BF16)
                        nc.sync.dma_start(wv[:], w_v.ap()[dvc])
                        for jc in range(JC):
                            acc = psacc.tile([P, 512], F32, tag="acc")
                            for k in range(DM):
                                nc.tensor.matmul(
                                    acc[:], xT[:, k, jc * P:(jc + 1) * P],
                                    wv[:, k, :],
                                    start=(k == 0), stop=(k == DM - 1))
                            nc.vector.tensor_copy(
                                vp_h[:, jc, dvc * 8:(dvc + 1) * 8, 0:64],
                                acc[:].rearrange("p (h e) -> p h e", e=64))

                kt = kt_pool.tile([P, DM, NSEQ], BF16, tag="kt")
                with tc.tile_pool(name="wk", bufs=2) as wk_pool:
                    for kf in range(DM):
                        wk = wk_pool.tile([P, DM, P], BF16)
                        nc.sync.dma_start(wk[:], w_k.ap()[kf])
                        for t in range(TC):
                            acc = psacc.tile([P, 512], F32, tag="acc")
                            for k in range(DM):
                                nc.tensor.matmul(
                                    acc[:], wk[:, k, :],
                                    xT[:, k, t * 512:(t + 1) * 512],
                                    start=(k == 0), stop=(k == DM - 1))
                            nc.vector.tensor_copy(
                                kt[:, kf, t * 512:(t + 1) * 512], acc[:])

            # -------- prefetch FFN weights during attention --------------
            w1ts, w2ts = [], []
            _w2cm = tc.tile_pool(name="w2p", bufs=4)
            w2_pool = _w2cm.__enter__()
            if "ffn" in phases:
                for fg in range(DFF // 512):
                    w1t = w1_pool.tile([P, DM, 512], BF16, tag="w1",
                                       name=f"w1t{fg}")
                    nc.gpsimd.dma_start(w1t[:], w1.ap()[fg])
                    w1ts.append(w1t)
                for ef in range(DM):
                    w2t = w2_pool.tile([P, FC, P], BF16, tag="w2",
                                       name=f"w2t{ef}")
                    nc.gpsimd.dma_start(w2t[:], w2.ap()[ef])
                    w2ts.append(w2t)

            # -------- attention (8 head-pairs) ---------------------------
            if "attn" not in phases:      # timing-bisect stub
                for k in range(DM):
                    nc.vector.tensor_copy(outT[:, k, :], QT[:, k, :])
            if "attn" in phases:
                with tc.tile_pool(name="pt", bufs=6) as pt_pool:
                    for hp in range(HPAIRS):
                        oacc = [psout.tile([65, NT], F32, tag="o",
                                           name=f"oacc{i}") for i in range(2)]
                        for jc in range(JC):
                            pt_ps = pspt.tile([P, 2 * NT], F32, tag="pt")
                            for i in range(2):
                                rows = slice(64 * i, 64 * i + 64)
                                nc.tensor.matmul(
                                    pt_ps[:, i * NT:(i + 1) * NT],
                                    kt[rows, hp, jc * P:(jc + 1) * P],
                                    QT[rows, hp, :],
                                    start=True, stop=True)
                            pt_sb = pt_pool.tile([P, 2 * NT], BF16, tag="ptsb")
                            nc.scalar.activation(pt_sb[:], pt_ps[:], AF.Exp,
                                                 scale=SCALE)
                            for i in range(2):
                                h = 2 * hp + i
                                nc.tensor.matmul(
                                    oacc[i][:],
                                    vp[:, jc, h * 65:(h + 1) * 65],
                                    pt_sb[:, i * NT:(i + 1) * NT],
                                    start=(jc == 0), stop=(jc == JC - 1))
                        for i in range(2):
                            rec = vec_pool.tile([1, NT], F32, tag="v")
                            nc.vector.reciprocal(rec[:], oacc[i][64:65, :])
                            bc = pspt.tile([64, NT], F32, tag="pt")
                            nc.tensor.matmul(bc[:], ones_row[:, 0:64], rec[:],
                                             start=True, stop=True)
                            bc_sb = sq_pool.tile([P, NT], F32, tag="sq")
                            nc.vector.tensor_copy(bc_sb[0:64, :], bc[:])
                            nc.vector.tensor_mul(
                                outT[64 * i:64 * i + 64, hp, :],
                                oacc[i][0:64, :], bc_sb[0:64, :])

            # -------- output projection + residual 1 ---------------------
            z1 = kt_pool.tile([P, DM, NT], F32, tag="kt")  # reuses kt slot
            with tc.tile_pool(name="wo", bufs=2) as wo_pool:
                for ef in range(DM):
                    wo = wo_pool.tile([P, DM, P], BF16)
                    nc.sync.dma_start(wo[:], w_out.ap()[ef])
                    acc = psacc.tile([P, NT], F32, tag="acc")
                    for k in range(DM):
                        nc.tensor.matmul(acc[:], wo[:, k, :], outT[:, k, :],
                                         start=(k == 0), stop=(k == DM - 1))
                    nc.vector.tensor_add(z1[:, ef, :], acc[:], xow[:, ef, :])

            # -------- LN1 ------------------------------------------------
            xln1f = vp_pool.tile([P, DM, NT], F32, tag="vp")  # reuses vp slot

            def write_xln1(k, t):
                nc.vector.tensor_scalar(xln1f[:, k, :], t[:],
                                        lnw1_sb[:, k:k + 1],
                                        lnb1_sb[:, k:k + 1],
                                        op0=mybir.AluOpType.mult,
                                        op1=mybir.AluOpType.add)
                nc.vector.tensor_copy(xln1[:, k, :], xln1f[:, k, :])
            ln_apply(z1, write_xln1)

            if "ffn" not in phases:   # timing-bisect stub: LN2 input
                z2 = pers.tile([P, DM, NT], F32, tag="tc")
                for k in range(DM):
                    nc.vector.tensor_copy(z2[:, k, :], z1[:, k, :])

            # -------- FFN ------------------------------------------------
            if "ffn" in phases:
                hT = kt_pool.tile([P, FC, NT], BF16, tag="kt")  # kt/z1 slot
                for fg in range(DFF // 512):
                    w1t = w1ts[fg]
                    for f4 in range(4):
                        f = fg * 4 + f4
                        acc = psacc.tile([P, NT], F32, tag="acc")
                        for k in range(DM):
                            nc.tensor.matmul(
                                acc[:], w1t[:, k, f4 * P:(f4 + 1) * P],
                                xln1[:, k, :],
                                start=(k == 0), stop=(k == DM - 1))
                        nc.scalar.activation(hT[:, f, :], acc[:], AF.Gelu,
                                             bias=b1_sb[:, f:f + 1])

                z2 = pers.tile([P, DM, NT], F32, tag="tc")  # xow slot
                for ef in range(DM):
                    w2t = w2ts[ef]
                    acc = psacc.tile([P, NT], F32, tag="acc")
                    for k in range(FC):
                        nc.tensor.matmul(acc[:], w2t[:, k, :], hT[:, k, :],
                                         start=(k == 0), stop=(k == FC - 1))
                    t = sq_pool.tile([P, NT], F32, tag="sq")
                    nc.vector.tensor_scalar_add(t[:], acc[:],
                                                b2_sb[:, ef:ef + 1])
                    nc.vector.tensor_add(z2[:, ef, :], t[:],
                                         xln1f[:, ef, :])
            _w2cm.__exit__(None, None, None)

        # -------- LN2 -> output ------------------------------------------
        with tc.tile_pool(name="outstage", bufs=2) as out_pool:
            yT_r = _rearr(yT_d.ap())

            def write_out(k, t):
                o = out_pool.tile([P, NT], F32)
                nc.vector.tensor_scalar(o[:], t[:],
                                        lnw2_sb[:, k:k + 1],
                                        lnb2_sb[:, k:k + 1],
                                        op0=mybir.AluOpType.mult,
                                        op1=mybir.AluOpType.add)
                nc.sync.dma_start(yT_r[:, k, :], o[:])
            ln_apply(z2, write_out)  # noqa: F821


def _get_nc():
    global _NC_CACHE
    if _NC_CACHE is None:
        _NC_CACHE = _build_nc()
    return _NC_CACHE


def _tile_w(W, out_cols):
    """[Din, Dout] f32 -> bf16 [Dout//out_cols, 128, Din//128, out_cols]
    so each output-chunk's weights are one contiguous DMA slab."""
    import ml_dtypes
    Din, Dout = W.shape
    t = W.astype(ml_dtypes.bfloat16).reshape(Din // P, P,
                                             Dout // out_cols, out_cols)
    return np.ascontiguousarray(t.transpose(2, 1, 0, 3))


def make_in_maps(x, w_qkv, w_out, ln1_w, ln1_b, w1, b1, w2, b2,
                 ln2_w, ln2_b):
    import ml_dtypes
    bf = ml_dtypes.bfloat16
    x = np.ascontiguousarray(np.asarray(x, dtype=np.float32))
    w_qkv = np.asarray(w_qkv, np.float32)
    shared = {
        "w_q": _tile_w(w_qkv[:, 0:D], P),
        "w_k": _tile_w(w_qkv[:, D:2 * D], P),
        "w_v": _tile_w(w_qkv[:, 2 * D:3 * D], 512),
        "w_out": _tile_w(np.asarray(w_out, np.float32), P),
        "w1": _tile_w(np.asarray(w1, np.float32), 512),
        "w2": _tile_w(np.asarray(w2, np.float32), P),
        "b1": np.asarray(b1, np.float32),
        "b2": np.asarray(b2, np.float32),
        "ln1_w": np.asarray(ln1_w, np.float32),
        "ln1_b": np.asarray(ln1_b, np.float32),
        "ln2_w": np.asarray(ln2_w, np.float32),
        "ln2_b": np.asarray(ln2_b, np.float32),
    }
    in_maps = []
    for c in range(8):
        b, q = divmod(c, 4)
        xT = np.ascontiguousarray(x[b].T)             # [D, NSEQ]
        # rotate so this core's own tokens are always columns [0, NT)
        xTr = np.ascontiguousarray(np.roll(xT, -q * NT, axis=1))
        in_maps.append({
            "xT": np.ascontiguousarray(xTr.astype(bf)),
            "x_own": np.ascontiguousarray(xTr[:, 0:NT]),
            **shared,
        })
    return in_maps


def kernel(x, w_qkv, w_out, ln1_w, ln1_b, w1, b1, w2, b2, ln2_w, ln2_b):
    in_maps = make_in_maps(x, w_qkv, w_out, ln1_w, ln1_b, w1, b1, w2, b2,
                           ln2_w, ln2_b)
    nc = _get_nc()
    res = run_bass_kernel_spmd(nc, in_maps, list(range(8)))

    out = np.empty((B, NSEQ, D), np.float32)
    for c in range(8):
        b, q = divmod(c, 4)
        out[b, q * NT:(q + 1) * NT, :] = res.results[c]["yT"].T
    return out



# revision 4
# speedup vs baseline: 1.0179x; 1.0179x over previous
"""Trainium2 Bass kernel for a single-layer transformer encoder.

Model: B=2, N=2048, D=1024, H=16, DFF=4096 (pre-computed QKV attention +
residual/LN + GELU FFN + residual/LN).

Sharding: 2 batches x 4-way sequence split (core c owns the 512 query tokens
q=c%4 of batch b=c//4). K/V projections are additionally PAIR-SPLIT: HBM on
trn2 is shared between core pairs (2k, 2k+1), so each core computes K/V for
only HALF its batch (rotated token quarters {0, 2}) and exchanges the other
half with its pair partner through a Shared-DRAM slab, synchronized by one
tiny AllGather barrier. Remote K/V land in separate tiles (ktr/vpr) so the
tile framework never serializes local attention on the exchange; reads are
ordered by consumption deadline (K-q1, V-q1, K-q3, V-q3) and the FFN SWDGE
prefetch is explicitly sequenced after the barrier so it cannot occupy the
gpsimd queue first.

All dense projections (QKV, attention output, FFN1/FFN2) run as fp8-e4m3
DoubleRow matmuls (2 contraction rows per PE cell); the attention
scores/AV matmuls and the kt/vp/QT operands stay bf16 (attention is
ScalarE-exp-bound, so fp8 would not help there), and the residual spine is
bf16 with fp32 PSUM accumulation everywhere.  Softmax runs on transposed
scores with denominators from a ones-column in V; exp runs on ScalarE over
[128, 1024] double-bank PSUM tiles.  Attention is scheduled in interleaved
head-pair groups (hpA locals, hpB locals, hpA remotes+fin, hpB remotes+fin)
with the two head-pairs' accumulators in different PSUM pools, giving the
K/V exchange a ~17us runway before the first remote chunk is needed.

LayerNorm stats (sum / sum-of-squares) are accumulated on the PE with bf16
ones-column matmuls interleaved into the producing loops (out-proj for LN1,
FFN2 for LN2); the apply phase splits token-halves across DVE and GpSimd
(GpSimd cannot touch PSUM, so broadcast R/MR tiles are staged to SBUF via
ScalarE copies, which also pre-loads the Sqrt activation table off the
critical path).  Output DMA is pipelined across both HWDGE queues.
"""

import os
import sys

for _p in ("/opt/trn_rl_repo", "/root/.axon_site", "/root/.axon_site/_ro/trn_rl_repo"):
    if os.path.isdir(_p) and _p not in sys.path:
        sys.path.append(_p)

import numpy as np

import concourse.bacc as bacc
import concourse.mybir as mybir
import concourse.tile as tile
from concourse.tile_rust import add_dep_helper
from concourse.bass_utils import run_bass_kernel_spmd

P = 128
B, NSEQ, D, H, DFF = 2, 2048, 1024, 16, 4096
DH = D // H                     # 64
NT = 512                        # query tokens per core
DM = D // P                     # 8 feature chunks
JC = NSEQ // P                  # 16 key-token chunks
TC = NSEQ // 512                # 4 512-token chunks
FC = DFF // P                   # 32 FFN feature chunks
HPAIRS = H // 2                 # 8
SCALE = DH ** -0.5
EPS = 1e-5

F32 = mybir.dt.float32
F32R = mybir.dt.float32r
BF16 = mybir.dt.bfloat16
FP8 = mybir.dt.float8e4
DR = mybir.MatmulPerfMode.DoubleRow
AF = mybir.ActivationFunctionType

# rotated key-chunk order: locally-computed quarters (0, 2) first, then the
# partner-provided quarters (1, 3)
JC_LOCAL = [0, 1, 2, 3, 8, 9, 10, 11]
JC_REMOTE = [4, 5, 6, 7, 12, 13, 14, 15]

KV_SPLIT = os.environ.get("KV_SPLIT", "1") == "1"
TWO_BARRIERS = os.environ.get("TWO_BARRIERS", "1") == "1"

_NC_CACHE = None


def _rearr(ap):
    """DRAM [D_like, T] -> [p, chunk, T] view with chunk-major features."""
    return ap.rearrange("(c p) t -> p c t", p=P)


def _build_nc(reps=1, phases=("qkv", "attn", "proj", "ffn")):
    nc = bacc.Bacc("TRN2", target_bir_lowering=False, debug=False)
    nc.num_devices = 8

    xT = nc.dram_tensor("xT", [D, NSEQ], FP8, kind="ExternalInput")
    x_own = nc.dram_tensor("x_own", [D, NT], BF16, kind="ExternalInput")
    # weights arrive pre-tiled: [out_chunk, partition, in_chunk, out_cols]
    w_q = nc.dram_tensor("w_q", [DM, P, DM, P], FP8, kind="ExternalInput")
    w_k = nc.dram_tensor("w_k", [DM, P, DM, P], FP8, kind="ExternalInput")
    w_v = nc.dram_tensor("w_v", [2, P, DM, 512], FP8, kind="ExternalInput")
    w_out = nc.dram_tensor("w_out", [DM, P, DM, P], FP8, kind="ExternalInput")
    w1 = nc.dram_tensor("w1", [DFF // 512, P, DM, 512], FP8,
                        kind="ExternalInput")
    w2 = nc.dram_tensor("w2", [DM, P, FC, P], FP8, kind="ExternalInput")
    b1 = nc.dram_tensor("b1", [DFF], F32, kind="ExternalInput")
    b2 = nc.dram_tensor("b2", [D], F32, kind="ExternalInput")
    ln1_w = nc.dram_tensor("ln1_w", [D], F32, kind="ExternalInput")
    ln1_b = nc.dram_tensor("ln1_b", [D], F32, kind="ExternalInput")
    ln2_w = nc.dram_tensor("ln2_w", [D], F32, kind="ExternalInput")
    ln2_b = nc.dram_tensor("ln2_b", [D], F32, kind="ExternalInput")
    yT = nc.dram_tensor("yT", [D, NT], F32, kind="ExternalOutput")

    # pair-shared K/V exchange slabs, flat-indexed [slot*2 + half]
    slab_k = nc.dram_tensor("slab_k", [4, P, DM, 512], BF16, kind="Internal",
                            addr_space="Shared")
    slab_v = nc.dram_tensor("slab_v", [4, P, 4, H, 65], BF16, kind="Internal",
                            addr_space="Shared")
    bar_in = nc.dram_tensor("bar_in", [1, 4], F32, kind="Internal")
    bar_k = nc.dram_tensor("bar_k", [2, 4], F32, kind="Internal")
    bar_v = nc.dram_tensor("bar_v", [2, 4], F32, kind="Internal")

    tensors = dict(xT=xT, x_own=x_own, w_q=w_q, w_k=w_k, w_v=w_v, w_out=w_out, w1=w1,
                   w2=w2, b1=b1, b2=b2, ln1_w=ln1_w, ln1_b=ln1_b,
                   ln2_w=ln2_w, ln2_b=ln2_b, yT=yT, slab_k=slab_k,
                   slab_v=slab_v, bar_in=bar_in, bar_k=bar_k, bar_v=bar_v)

    with tile.TileContext(nc) as tc, \
         nc.allow_low_precision(reason="bf16 matmul operands; fp32 spine"):
        for r in range(reps):
            _emit(nc, tc, tensors, phases=phases)
    nc.compile()
    return nc


def _emit(nc, tc, T, phases=("qkv", "attn", "proj", "ffn")):
    xT_d, yT_d = T["xT"], T["yT"]
    slab_k, slab_v = T["slab_k"], T["slab_v"]

    pid = nc.partition_id()     # multi-engine: used from sync/scalar/gpsimd
    slot = pid & 1              # 0 on even cores, 1 on odd
    other = (pid + 1) & 1

    # ---------------- whole-kernel pools ----------------
    with tc.tile_pool(name="const", bufs=1) as pc, \
         tc.tile_pool(name="pers", bufs=1) as pers, \
         tc.tile_pool(name="scratch", bufs=3) as sq_pool, \
         tc.tile_pool(name="vecs", bufs=5) as vec_pool, \
         tc.tile_pool(name="psacc", bufs=2, space="PSUM") as psacc, \
         tc.tile_pool(name="pspt", bufs=2, space="PSUM") as pspt, \
         tc.tile_pool(name="psout", bufs=2, space="PSUM") as psout:

        # ---------------- constants (scalar queue) ----------------
        ones_f32 = pc.tile([P, 2 * P], F32)
        nc.vector.memset(ones_f32[:], 1.0)
        ones_col_r = ones_f32[:, 0:1].bitcast(F32R)
        ones_col = pc.tile([P, 1], BF16)          # lhsT for bf16 stat sums
        nc.vector.tensor_copy(ones_col[:], ones_f32[:, 0:1])
        ones_row = pc.tile([1, P], F32)           # lhsT for exact broadcasts
        nc.vector.tensor_copy(ones_row[:], ones_f32[0:1, 0:P])
        eps_sb = pc.tile([1, 1], F32)
        nc.vector.memset(eps_sb[:], EPS)
        b1_sb = pc.tile([P, FC], F32)
        b2_sb = pc.tile([P, DM], F32)
        lnw1_sb = pc.tile([P, DM], F32)
        lnb1_sb = pc.tile([P, DM], F32)
        lnw2_sb = pc.tile([P, DM], F32)
        lnb2_sb = pc.tile([P, DM], F32)
        bi = pc.tile([1, 4], F32)
        nc.vector.memset(bi[:], 1.0)
        dummy = pc.tile([1, 1], F32)
        nc.vector.memset(dummy[:], 1.0)

        def emit_const_dmas():
            nc.scalar.dma_start(T["bar_in"].ap(), bi[:])
            for sb, t in ((b1_sb, "b1"), (b2_sb, "b2"),
                          (lnw1_sb, "ln1_w"), (lnb1_sb, "ln1_b"),
                          (lnw2_sb, "ln2_w"), (lnb2_sb, "ln2_b")):
                nc.scalar.dma_start(
                    sb[:], T[t].ap().rearrange("(c p) -> p c", p=P))

        # persistent activations
        QT = pers.tile([P, DM, NT], BF16)
        outT = pers.tile([P, DM, NT], FP8)
        xow = pers.tile([P, DM, NT], BF16, tag="tc")
        xln18 = pers.tile([P, DM, NT], FP8)  # fp8 operand copy of xln1  # own-token x (residual 1)
        xln1 = pers.tile([P, DM, NT], BF16)     # LN1 out (ffn operand+residual)

        def ln_apply(z_tile, writes, interleave=None):
            """LayerNorm over features of z_tile [P, DM, NT] (fp32).
            writes(k, src_f32_ap) stores chunk k.  If interleave is given,
            (s1, s2) stats were already accumulated there by the caller."""
            if interleave is None:
                s1 = psacc.tile([1, NT], F32, tag="acc")
                s2 = psacc.tile([1, NT], F32, tag="acc")
                for k in range(DM):
                    eng = nc.vector if k % 2 == 0 else nc.gpsimd
                    nc.tensor.matmul(s1[:], ones_col[:], z_tile[:, k, :],
                                     start=(k == 0), stop=(k == DM - 1))
                    sq = sq_pool.tile([P, NT], BF16, tag="sq")
                    eng.tensor_mul(sq[:], z_tile[:, k, :], z_tile[:, k, :])
                    nc.tensor.matmul(s2[:], ones_col[:], sq[:],
                                     start=(k == 0), stop=(k == DM - 1))
            else:
                s1, s2 = interleave
            mu = vec_pool.tile([1, NT], F32, tag="v")
            nc.vector.tensor_scalar_mul(mu[:], s1[:], 1.0 / D)
            var = vec_pool.tile([1, NT], F32, tag="v")
            nc.vector.tensor_scalar_mul(var[:], s2[:], 1.0 / D)
            musq = vec_pool.tile([1, NT], F32, tag="v")
            nc.vector.tensor_mul(musq[:], mu[:], mu[:])
            nc.vector.tensor_sub(var[:], var[:], musq[:])
            nc.scalar.activation(var[:], var[:], AF.Sqrt, bias=eps_sb[:])
            rec = vec_pool.tile([1, NT], F32, tag="v")
            nc.vector.reciprocal(rec[:], var[:])
            murf = vec_pool.tile([1, NT], F32, tag="v")
            nc.vector.tensor_mul(murf[:], mu[:], rec[:])
            R = psacc.tile([P, NT], F32, tag="acc")
            nc.tensor.matmul(R[:], ones_row[:], rec[:], start=True, stop=True)
            MR = psacc.tile([P, NT], F32, tag="acc")
            nc.tensor.matmul(MR[:], ones_row[:], murf[:], start=True, stop=True)
            R_sb = vec_pool.tile([P, NT], F32, tag="v")
            nc.scalar.copy(R_sb[:], R[:])
            MR_sb = vec_pool.tile([P, NT], F32, tag="v")
            nc.scalar.copy(MR_sb[:], MR[:])
            HNT = NT // 2
            for k in range(DM):
                for h in range(2):
                    eng = nc.vector if h == 0 else nc.gpsimd
                    cols = slice(h * HNT, (h + 1) * HNT)
                    t = sq_pool.tile([P, HNT], F32, tag="sq")
                    eng.tensor_mul(t[:], z_tile[:, k, cols], R_sb[:, cols])
                    eng.tensor_sub(t[:], t[:], MR_sb[:, cols])
                    writes(k, cols, t, eng)

        with tc.tile_pool(name="ktp", bufs=1) as kt_pool, \
             tc.tile_pool(name="vpp", bufs=1) as vp_pool:

            kt = kt_pool.tile([P, DM, 1024], BF16, tag="kt")
            ktr = kt_pool.tile([P, DM, 1024], BF16, tag="ktr")
            vp = vp_pool.tile([P, 8, H * 65], BF16, tag="vp")
            vpr = vp_pool.tile([P, 8, H * 65], BF16, tag="vpr")
            vp_h = vp.rearrange("p j (h e) -> p j h e", e=65)
            vpr_h = vpr.rearrange("p j (h e) -> p j h e", e=65)
            nc.vector.tensor_copy(
                vp_h[:, :, :, 64:65],
                ones_f32[:, 0:128].rearrange("p (a b c) -> p a b c", b=H, c=1))
            if not KV_SPLIT:
                nc.vector.tensor_copy(
                    vpr_h[:, :, :, 64:65],
                    ones_f32[:, 0:128].rearrange("p (a b c) -> p a b c",
                                                 b=H, c=1))

            def kt_at(jc):
                """(tile, column-base) for rotated key chunk jc."""
                if jc < 4: return kt, jc * P
                if jc < 8: return ktr, (jc - 4) * P
                if jc < 12: return kt, (jc - 4) * P
                return ktr, (jc - 8) * P

            def vp_at(jc):
                if jc < 4: return vp, jc
                if jc < 8: return vpr, jc - 4
                if jc < 12: return vp, jc - 4
                return vpr, jc - 8

            def vp_at_h(jc):
                if jc < 4: return vp_h, jc
                if jc < 8: return vpr_h, jc - 4
                if jc < 12: return vp_h, jc - 4
                return vpr_h, jc - 8

            kw_insts, vw_insts = [], []

            with tc.tile_pool(name="xpool", bufs=1) as px:
                xT = px.tile([P, DM, NSEQ], FP8)
                xTs = _rearr(xT_d.ap())
                xTq = xTs.rearrange("p c (h q2 t) -> p c h q2 t", h=2, q2=2)
                xTt = xT.rearrange("p c (h q2 t) -> p c h q2 t", h=2, q2=2)
                # own/local quarters first so K/V of quarters {0,2} can start
                nc.sync.dma_start(xTt[:, :, 0, 0, :], xTq[:, :, 0, 0, :])
                nc.sync.dma_start(xTt[:, :, 1, 0, :], xTq[:, :, 1, 0, :])
                nc.sync.dma_start(xTt[:, :, 0, 1, :], xTq[:, :, 0, 1, :])
                nc.sync.dma_start(xTt[:, :, 1, 1, :], xTq[:, :, 1, 1, :])

                with tc.tile_pool(name="wall", bufs=1) as wall_pool:
                    wk = wall_pool.tile([P, DM, DM, P], FP8, tag="wk")
                    wks = T["w_k"].ap().rearrange("f p k t -> p f k t")
                    nc.scalar.dma_start(wk[:, 0:4], wks[:, 0:4])
                    nc.scalar.dma_start(wk[:, 4:8], wks[:, 4:8])
                    wv = wall_pool.tile([P, 2, DM, 512], FP8, tag="wv")
                    wvs = T["w_v"].ap().rearrange("f p k t -> p f k t")
                    nc.scalar.dma_start(wv[:, 0:1], wvs[:, 0:1])
                    nc.scalar.dma_start(wv[:, 1:2], wvs[:, 1:2])
                    wq = wall_pool.tile([P, DM, DM, P], FP8, tag="wq")
                    wqs = T["w_q"].ap().rearrange("f p k t -> p f k t")
                    nc.scalar.dma_start(wq[:, 0:4], wqs[:, 0:4])
                    nc.scalar.dma_start(wq[:, 4:8], wqs[:, 4:8])
                    emit_const_dmas()
                    last_xow_cp = nc.scalar.dma_start(
                        xow[:], _rearr(T["x_own"].ap()))

                    # -------- K projection --------------------------------
                    for kf in range(DM):
                        for t in ((0, 2) if KV_SPLIT else (0, 1, 2, 3)):
                            acc = psacc.tile([P, 512], F32, tag="acc")
                            for k in range(DM // 2):
                                nc.tensor.matmul(
                                    acc[:], wk[:, kf, 2 * k:2 * k + 2, :],
                                    xT[:, 2 * k:2 * k + 2,
                                       t * 512:(t + 1) * 512],
                                    start=(k == 0), stop=(k == DM // 2 - 1),
                                    perf_mode=DR)
                            dst, cb = kt_at(t * 4)
                            nc.vector.tensor_copy(
                                dst[:, kf, cb:cb + 512], acc[:])
                            if KV_SPLIT:
                                w = nc.sync.dma_start(
                                    slab_k.ap()[slot * 2 + t // 2][:, kf, :],
                                    dst[:, kf, cb:cb + 512])
                                kw_insts.append(w)

                    if KV_SPLIT and "attn" in phases and TWO_BARRIERS:
                        cck = nc.gpsimd.collective_compute(
                            "AllGather", mybir.AluOpType.bypass,
                            ins=[T["bar_in"].ap()], outs=[T["bar_k"].ap()],
                            replica_groups=[[0, 1], [2, 3], [4, 5], [6, 7]])
                        for w in kw_insts:
                            add_dep_helper(cck.ins, w.ins,
                                           reason="barrier after K writes")
                        rk1 = nc.scalar.dma_start(ktr[:, :, 0:512],
                                                slab_k.ap()[other * 2 + slot])
                        rk3 = nc.scalar.dma_start(ktr[:, :, 512:1024],
                                                slab_k.ap()[other * 2 + 1 - slot])
                        add_dep_helper(rk1.ins, cck.ins, reason="K read after barrier")
                        add_dep_helper(rk3.ins, cck.ins, reason="K read after barrier")
                        cc_last = cck

                    # -------- V projection --------------------------------
                    for dvc in range(2):  # 512 v-features = 8 heads at a time
                        for half in range(2 if KV_SPLIT else 4):
                            for jc4 in range(4):
                                jc = (half * 8 + jc4) if KV_SPLIT \
                                    else (half * 4 + jc4)
                                acc = psacc.tile([P, 512], F32, tag="acc")
                                for k in range(DM // 2):
                                    nc.tensor.matmul(
                                        acc[:],
                                        xT[:, 2 * k:2 * k + 2,
                                           jc * P:(jc + 1) * P],
                                        wv[:, dvc, 2 * k:2 * k + 2, :],
                                        start=(k == 0),
                                        stop=(k == DM // 2 - 1),
                                        perf_mode=DR)
                                dvh, jpos = vp_at_h(jc)
                                nc.vector.tensor_copy(
                                    dvh[:, jpos, dvc * 8:(dvc + 1) * 8, 0:64],
                                    acc[:].rearrange("p (h e) -> p h e", e=64))
                            if KV_SPLIT:
                                w = nc.sync.dma_start(
                                    slab_v.ap()[slot * 2 + half]
                                    [:, :, dvc * 8:(dvc + 1) * 8, :],
                                    vp_h[:, half * 4:half * 4 + 4,
                                         dvc * 8:(dvc + 1) * 8, 0:65])
                                vw_insts.append(w)

                    if KV_SPLIT and "attn" in phases:
                        ccv = nc.gpsimd.collective_compute(
                            "AllGather", mybir.AluOpType.bypass,
                            ins=[T["bar_in"].ap()], outs=[T["bar_v"].ap()],
                            replica_groups=[[0, 1], [2, 3], [4, 5], [6, 7]])
                        for w in vw_insts:
                            add_dep_helper(ccv.ins, w.ins,
                                           reason="barrier after V writes")
                        if not TWO_BARRIERS:
                            for w in kw_insts:
                                add_dep_helper(ccv.ins, w.ins,
                                               reason="barrier after K writes")
                            rk1 = nc.scalar.dma_start(ktr[:, :, 0:512],
                                                    slab_k.ap()[other * 2 + slot])
                            add_dep_helper(rk1.ins, ccv.ins,
                                           reason="K read after barrier")
                        rv1 = nc.scalar.dma_start(
                            vpr_h[:, 0:4, :, 0:65], slab_v.ap()[other * 2 + slot])
                        add_dep_helper(rv1.ins, ccv.ins, reason="V read after barrier")
                        if not TWO_BARRIERS:
                            rk3 = nc.scalar.dma_start(
                                ktr[:, :, 512:1024],
                                slab_k.ap()[other * 2 + 1 - slot])
                            add_dep_helper(rk3.ins, ccv.ins,
                                           reason="K read after barrier")
                        rv3 = nc.scalar.dma_start(
                            vpr_h[:, 4:8, :, 0:65],
                            slab_v.ap()[other * 2 + 1 - slot])
                        add_dep_helper(rv3.ins, ccv.ins, reason="V read after barrier")
                        cc_last = ccv

                    # -------- Q projection (own tokens) -------------------
                    for qf in range(DM):
                        acc = psacc.tile([P, NT], F32, tag="acc")
                        for k in range(DM // 2):
                            last_q_mm = nc.tensor.matmul(
                                acc[:], wq[:, qf, 2 * k:2 * k + 2, :],
                                xT[:, 2 * k:2 * k + 2, 0:NT],
                                start=(k == 0), stop=(k == DM // 2 - 1),
                                perf_mode=DR)
                        nc.vector.tensor_copy(QT[:, qf, :], acc[:])

            # -------- prefetch FFN weights (SWDGE; xT slot now free) -----
            _w1cm = tc.tile_pool(name="w1p", bufs=4)
            w1_pool = _w1cm.__enter__()
            _w2cm = tc.tile_pool(name="w2p", bufs=4)
            w2_pool = _w2cm.__enter__()
            w1ts, w2ts = [], []
            if "ffn" in phases:
                for fg in range(DFF // 512):
                    w1t = w1_pool.tile([P, DM, 512], FP8, tag="w1",
                                       name=f"w1t{fg}")
                    d = nc.gpsimd.dma_start(w1t[:], T["w1"].ap()[fg])
                    add_dep_helper(d.ins, last_q_mm.ins,
                                   reason="w1 slab reuses phase-1 SBUF")
                    if KV_SPLIT and "attn" in phases:
                        add_dep_helper(d.ins, cc_last.ins,
                                       reason="barrier owns Pool queue first")
                    add_dep_helper(d.ins, last_xow_cp.ins,
                                   reason="w1 slab reuses phase-1 SBUF")
                    w1ts.append(w1t)
                for ef in range(DM):
                    w2t = w2_pool.tile([P, FC, P], FP8, tag="w2",
                                       name=f"w2t{ef}")
                    d = nc.gpsimd.dma_start(w2t[:], T["w2"].ap()[ef])
                    add_dep_helper(d.ins, last_q_mm.ins,
                                   reason="w2 slab reuses phase-1 SBUF")
                    if KV_SPLIT and "attn" in phases:
                        add_dep_helper(d.ins, cc_last.ins,
                                       reason="barrier owns Pool queue first")
                    add_dep_helper(d.ins, last_xow_cp.ins,
                                   reason="w2 slab reuses phase-1 SBUF")
                    w2ts.append(w2t)

            # -------- attention (interleaved head-pair groups) ----------
            if "attn" not in phases:      # timing-bisect stub
                for k in range(DM):
                    nc.vector.tensor_copy(outT[:, k, :], QT[:, k, :])
            if "attn" in phases:
                with tc.tile_pool(name="pt", bufs=5) as pt_pool:
                    oaccs = {}
                    first_exps = []

                    def attn_chunk(hp, jcs, pool, tag, start, stop):
                        if start:
                            oaccs[hp] = [pool.tile([65, NT], F32, tag=tag,
                                                   name=f"oacc{hp}_{i}")
                                         for i in range(2)]
                        oacc = oaccs[hp]
                        for n, jc in enumerate(jcs):
                            ksrc, kcb = kt_at(jc)
                            vsrc, vpos = vp_at(jc)
                            pt_ps = pspt.tile([P, 2 * NT], F32, tag="pt")
                            for i in range(2):
                                rows = slice(64 * i, 64 * i + 64)
                                nc.tensor.matmul(
                                    pt_ps[:, i * NT:(i + 1) * NT],
                                    ksrc[rows, hp, kcb:kcb + P],
                                    QT[rows, hp, :],
                                    start=True, stop=True)
                            pt_sb = pt_pool.tile([P, 2 * NT], BF16, tag="ptsb")
                            e = nc.scalar.activation(pt_sb[:], pt_ps[:], AF.Exp,
                                                     scale=SCALE)
                            if len(first_exps) < 6:
                                first_exps.append(e)
                                add_dep_helper(e.ins, last_q_mm.ins,
                                               reason="pt_sb reuses phase-1 SBUF")
                            for i in range(2):
                                h = 2 * hp + i
                                nc.tensor.matmul(
                                    oacc[i][:],
                                    vsrc[:, vpos, h * 65:(h + 1) * 65],
                                    pt_sb[:, i * NT:(i + 1) * NT],
                                    start=(start and n == 0),
                                    stop=(stop and n == len(jcs) - 1))

                    def attn_fin(hp):
                        oacc = oaccs.pop(hp)
                        bc2 = pspt.tile([P, NT], F32, tag="pt")
                        for i in range(2):
                            rec = vec_pool.tile([1, NT], F32, tag="v")
                            nc.vector.reciprocal(rec[:], oacc[i][64:65, :])
                            nc.tensor.matmul(bc2[64 * i:64 * i + 64, :],
                                             ones_row[:, 0:64], rec[:],
                                             start=True, stop=True)
                        bc_sb = sq_pool.tile([P, NT], F32, tag="sq")
                        nc.scalar.copy(bc_sb[:], bc2[:])
                        for i in range(2):
                            nc.vector.tensor_mul(
                                outT[64 * i:64 * i + 64, hp, :],
                                oacc[i][0:64, :],
                                bc_sb[64 * i:64 * i + 64, :])

                    for hpp in range(HPAIRS // 2):
                        hpA, hpB = 2 * hpp, 2 * hpp + 1
                        attn_chunk(hpA, JC_LOCAL, psout, "o", True, False)
                        attn_chunk(hpB, JC_LOCAL, psacc, "acc", True, False)
                        attn_chunk(hpA, JC_REMOTE, psout, "o", False, True)
                        attn_fin(hpA)
                        attn_chunk(hpB, JC_REMOTE, psacc, "acc", False, True)
                        attn_fin(hpB)

            # -------- output projection + residual 1 + LN1 stats ---------
            nc.scalar.activation(dummy[:], dummy[:], AF.Sqrt)  # table preload
            z1 = kt_pool.tile([P, DM, NT], BF16, tag="kt")  # reuses kt slot
            s1 = psacc.tile([1, NT], F32, tag="acc")
            s2 = psacc.tile([1, NT], F32, tag="acc")
            with tc.tile_pool(name="wo", bufs=2) as wo_pool:
                for ef in range(DM):
                    wo = wo_pool.tile([P, DM, P], FP8)
                    nc.sync.dma_start(wo[:], T["w_out"].ap()[ef])
                    acc = pspt.tile([P, NT], F32, tag="pt")
                    for k in range(DM // 2):
                        nc.tensor.matmul(acc[:], wo[:, 2 * k:2 * k + 2, :],
                                         outT[:, 2 * k:2 * k + 2, :],
                                         start=(k == 0),
                                         stop=(k == DM // 2 - 1),
                                         perf_mode=DR)
                    nc.vector.tensor_add(z1[:, ef, :], acc[:], xow[:, ef, :])
                    nc.tensor.matmul(s1[:], ones_col[:], z1[:, ef, :],
                                     start=(ef == 0), stop=(ef == DM - 1))
                    sq = sq_pool.tile([P, NT], BF16, tag="sq")
                    nc.gpsimd.tensor_mul(sq[:], z1[:, ef, :], z1[:, ef, :])
                    nc.tensor.matmul(s2[:], ones_col[:], sq[:],
                                     start=(ef == 0), stop=(ef == DM - 1))

            # -------- LN1 ------------------------------------------------
            def write_xln1(k, cols, t, eng):
                eng.tensor_scalar(xln1[:, k, cols], t[:],
                                  lnw1_sb[:, k:k + 1],
                                  lnb1_sb[:, k:k + 1],
                                  op0=mybir.AluOpType.mult,
                                  op1=mybir.AluOpType.add)
                nc.scalar.copy(xln18[:, k, cols], xln1[:, k, cols])
            ln_apply(z1, write_xln1, interleave=(s1, s2))

            if "ffn" not in phases:   # timing-bisect stub: LN2 input
                z2 = pers.tile([P, DM, NT], BF16, tag="tc")  # xow slot
                s1 = psacc.tile([1, NT], F32, tag="acc")
                s2 = psacc.tile([1, NT], F32, tag="acc")
                for k in range(DM):
                    eng = nc.vector if k % 2 == 0 else nc.gpsimd
                    nc.vector.tensor_copy(z2[:, k, :], z1[:, k, :])
                    nc.tensor.matmul(s1[:], ones_col[:], z2[:, k, :],
                                     start=(k == 0), stop=(k == DM - 1))
                    sq = sq_pool.tile([P, NT], BF16, tag="sq")
                    eng.tensor_mul(sq[:], z2[:, k, :], z2[:, k, :])
                    nc.tensor.matmul(s2[:], ones_col[:], sq[:],
                                     start=(k == 0), stop=(k == DM - 1))

            # -------- FFN ------------------------------------------------
            if "ffn" in phases:
                hT = kt_pool.tile([P, FC, NT], FP8, tag="kt")  # kt/z1 slot
                for fg in range(DFF // 512):
                    w1t = w1ts[fg]
                    for f4 in range(4):
                        f = fg * 4 + f4
                        acc = psacc.tile([P, NT], F32, tag="acc")
                        for k in range(DM // 2):
                            nc.tensor.matmul(
                                acc[:],
                                w1t[:, 2 * k:2 * k + 2, f4 * P:(f4 + 1) * P],
                                xln18[:, 2 * k:2 * k + 2, :],
                                start=(k == 0), stop=(k == DM // 2 - 1),
                                perf_mode=DR)
                        nc.scalar.activation(hT[:, f, :], acc[:], AF.Gelu,
                                             bias=b1_sb[:, f:f + 1])

                # FFN2 with LN2 stats interleaved (stats in psacc, accs in pspt)
                nc.scalar.activation(dummy[:], dummy[:], AF.Sqrt)  # preload
                z2 = pers.tile([P, DM, NT], BF16, tag="tc")  # xow slot
                s1 = psacc.tile([1, NT], F32, tag="acc")
                s2 = psacc.tile([1, NT], F32, tag="acc")
                for ef in range(DM):
                    w2t = w2ts[ef]
                    acc = pspt.tile([P, NT], F32, tag="pt")
                    for k in range(FC // 2):
                        nc.tensor.matmul(acc[:], w2t[:, 2 * k:2 * k + 2, :],
                                         hT[:, 2 * k:2 * k + 2, :],
                                         start=(k == 0),
                                         stop=(k == FC // 2 - 1),
                                         perf_mode=DR)
                    eng = nc.vector if ef % 2 == 0 else nc.gpsimd
                    t = sq_pool.tile([P, NT], F32, tag="sq")
                    nc.scalar.activation(t[:], acc[:], AF.Identity,
                                         bias=b2_sb[:, ef:ef + 1])
                    nc.vector.tensor_add(z2[:, ef, :], t[:], xln1[:, ef, :])
                    nc.tensor.matmul(s1[:], ones_col[:], z2[:, ef, :],
                                     start=(ef == 0), stop=(ef == DM - 1))
                    sq = sq_pool.tile([P, NT], BF16, tag="sq")
                    eng.tensor_mul(sq[:], z2[:, ef, :], z2[:, ef, :])
                    nc.tensor.matmul(s2[:], ones_col[:], sq[:],
                                     start=(ef == 0), stop=(ef == DM - 1))
            _w2cm.__exit__(None, None, None)
            _w1cm.__exit__(None, None, None)

        # -------- LN2 -> output ------------------------------------------
        with tc.tile_pool(name="outstage", bufs=4) as out_pool:
            yT_r = _rearr(yT_d.ap())

            def write_out(k, cols, t, eng):
                o = out_pool.tile([P, NT // 2], F32)
                eng.tensor_scalar(o[:], t[:],
                                  lnw2_sb[:, k:k + 1],
                                  lnb2_sb[:, k:k + 1],
                                  op0=mybir.AluOpType.mult,
                                  op1=mybir.AluOpType.add)
                q = nc.sync if k % 2 == 0 else nc.scalar
                q.dma_start(yT_r[:, k, cols], o[:])
            ln_apply(z2, write_out, interleave=(s1, s2))  # noqa: F821


def _get_nc():
    global _NC_CACHE
    if _NC_CACHE is None:
        _NC_CACHE = _build_nc()
    return _NC_CACHE


def _tile_w(W, out_cols):
    """[Din, Dout] f32 -> fp8 [Dout//out_cols, 128, Din//128, out_cols]
    so each output-chunk's weights are one contiguous DMA slab."""
    f8 = mybir.dt.np(FP8)
    Din, Dout = W.shape
    t = W.astype(f8).reshape(Din // P, P, Dout // out_cols, out_cols)
    return np.ascontiguousarray(t.transpose(2, 1, 0, 3))


def make_in_maps(x, w_qkv, w_out, ln1_w, ln1_b, w1, b1, w2, b2,
                 ln2_w, ln2_b):
    import ml_dtypes
    bf = ml_dtypes.bfloat16
    x = np.ascontiguousarray(np.asarray(x, dtype=np.float32))
    w_qkv = np.asarray(w_qkv, np.float32)
    shared = {
        "w_q": _tile_w(w_qkv[:, 0:D], P),
        "w_k": _tile_w(w_qkv[:, D:2 * D], P),
        "w_v": _tile_w(w_qkv[:, 2 * D:3 * D], 512),
        "w_out": _tile_w(np.asarray(w_out, np.float32), P),
        "w1": _tile_w(np.asarray(w1, np.float32), 512),
        "w2": _tile_w(np.asarray(w2, np.float32), P),
        "b1": np.asarray(b1, np.float32),
        "b2": np.asarray(b2, np.float32),
        "ln1_w": np.asarray(ln1_w, np.float32),
        "ln1_b": np.asarray(ln1_b, np.float32),
        "ln2_w": np.asarray(ln2_w, np.float32),
        "ln2_b": np.asarray(ln2_b, np.float32),
    }
    f8 = mybir.dt.np(FP8)
    in_maps = []
    for c in range(8):
        b, q = divmod(c, 4)
        xT = np.ascontiguousarray(x[b].T)             # [D, NSEQ]
        # rotate so this core's own tokens are always columns [0, NT)
        xTr = np.ascontiguousarray(np.roll(xT, -q * NT, axis=1))
        in_maps.append({
            "xT": np.ascontiguousarray(xTr.astype(f8)),
            "x_own": np.ascontiguousarray(xTr[:, 0:NT].astype(bf)),
            **shared,
        })
    return in_maps


def kernel(x, w_qkv, w_out, ln1_w, ln1_b, w1, b1, w2, b2, ln2_w, ln2_b):
    in_maps = make_in_maps(x, w_qkv, w_out, ln1_w, ln1_b, w1, b1, w2, b2,
                           ln2_w, ln2_b)
    nc = _get_nc()
    res = run_bass_kernel_spmd(nc, in_maps, list(range(8)))

    out = np.empty((B, NSEQ, D), np.float32)
    for c in range(8):
        b, q = divmod(c, 4)
        out[b, q * NT:(q + 1) * NT, :] = res.results[c]["yT"].T
    return out


# revision 5
# speedup vs baseline: 1.1272x; 1.1074x over previous
"""Trainium2 Bass kernel for a single-layer transformer encoder.

Model: B=2, N=2048, D=1024, H=16, DFF=4096 (pre-computed QKV attention +
residual/LN + GELU FFN + residual/LN).

Sharding: 2 batches x 4-way sequence split (core c owns the 512 query tokens
q=c%4 of batch b=c//4). K/V projections are additionally PAIR-SPLIT: HBM on
trn2 is shared between core pairs (2k, 2k+1), so each core computes K/V for
only HALF its batch (rotated token quarters {0, 2}) and exchanges the other
half with its pair partner through a Shared-DRAM slab, synchronized by one
tiny AllGather barrier. Remote K/V land in separate tiles (ktr/vpr) so the
tile framework never serializes local attention on the exchange; reads are
ordered by consumption deadline (K-q1, V-q1, K-q3, V-q3) and the FFN SWDGE
prefetch is explicitly sequenced after the barrier so it cannot occupy the
gpsimd queue first.

All dense projections (QKV, attention output, FFN1/FFN2) run as fp8-e4m3
DoubleRow matmuls (2 contraction rows per PE cell); the attention
scores/AV matmuls and the kt/vp/QT operands stay bf16 (attention is
ScalarE-exp-bound, so fp8 would not help there), and the residual spine is
bf16 with fp32 PSUM accumulation everywhere.  Softmax runs on transposed
scores with denominators from a ones-column in V; exp runs on ScalarE over
[128, 1024] double-bank PSUM tiles.  Attention is scheduled in interleaved
head-pair groups (hpA locals, hpB locals, hpA remotes+fin, hpB remotes+fin)
with the two head-pairs' accumulators in different PSUM pools, giving the
K/V exchange a ~17us runway before the first remote chunk is needed.

LayerNorm stats (sum / sum-of-squares) are accumulated on the PE with bf16
ones-column matmuls interleaved into the producing loops (out-proj for LN1,
FFN2 for LN2); the apply phase splits token-halves across DVE and GpSimd
(GpSimd cannot touch PSUM, so broadcast R/MR tiles are staged to SBUF via
ScalarE copies, which also pre-loads the Sqrt activation table off the
critical path).  Output DMA is pipelined across both HWDGE queues.
"""

import os
import sys

for _p in ("/opt/trn_rl_repo", "/root/.axon_site", "/root/.axon_site/_ro/trn_rl_repo"):
    if os.path.isdir(_p) and _p not in sys.path:
        sys.path.append(_p)

import numpy as np

import concourse.bacc as bacc
import concourse.mybir as mybir
import concourse.tile as tile
from concourse.tile_rust import add_dep_helper
from concourse.bass_utils import run_bass_kernel_spmd

P = 128
B, NSEQ, D, H, DFF = 2, 2048, 1024, 16, 4096
DH = D // H                     # 64
NT = 512                        # query tokens per core
DM = D // P                     # 8 feature chunks
JC = NSEQ // P                  # 16 key-token chunks
TC = NSEQ // 512                # 4 512-token chunks
FC = DFF // P                   # 32 FFN feature chunks
HPAIRS = H // 2                 # 8
SCALE = DH ** -0.5
EPS = 1e-5

F32 = mybir.dt.float32
F32R = mybir.dt.float32r
BF16 = mybir.dt.bfloat16
FP8 = mybir.dt.float8e4
DR = mybir.MatmulPerfMode.DoubleRow
AF = mybir.ActivationFunctionType

# rotated key-chunk order: locally-computed quarters (0, 2) first, then the
# partner-provided quarters (1, 3)
JC_LOCAL = [0, 1, 2, 3, 8, 9, 10, 11]
JC_REMOTE = [4, 5, 6, 7, 12, 13, 14, 15]

KV_SPLIT = os.environ.get("KV_SPLIT", "1") == "1"
TWO_BARRIERS = os.environ.get("TWO_BARRIERS", "1") == "1"

_NC_CACHE = None


def _rearr(ap):
    """DRAM [D_like, T] -> [p, chunk, T] view with chunk-major features."""
    return ap.rearrange("(c p) t -> p c t", p=P)


def _build_nc(reps=1, phases=("qkv", "attn", "proj", "ffn")):
    nc = bacc.Bacc("TRN2", target_bir_lowering=False, debug=False)
    nc.num_devices = 8

    xT = nc.dram_tensor("xT", [D, NSEQ], FP8, kind="ExternalInput")
    x_own = nc.dram_tensor("x_own", [D, NT], BF16, kind="ExternalInput")
    # weights arrive pre-tiled: [out_chunk, partition, in_chunk, out_cols]
    w_q = nc.dram_tensor("w_q", [DM, P, DM, P], FP8, kind="ExternalInput")
    w_k = nc.dram_tensor("w_k", [DM, P, DM, P], FP8, kind="ExternalInput")
    w_v = nc.dram_tensor("w_v", [2, P, DM, 512], FP8, kind="ExternalInput")
    w_out = nc.dram_tensor("w_out", [DM, P, DM, P], FP8, kind="ExternalInput")
    w1 = nc.dram_tensor("w1", [DFF // 512, P, DM, 512], FP8,
                        kind="ExternalInput")
    w2 = nc.dram_tensor("w2", [DM, P, FC, P], FP8, kind="ExternalInput")
    b1 = nc.dram_tensor("b1", [DFF], F32, kind="ExternalInput")
    b2 = nc.dram_tensor("b2", [D], F32, kind="ExternalInput")
    ln1_w = nc.dram_tensor("ln1_w", [D], F32, kind="ExternalInput")
    ln1_b = nc.dram_tensor("ln1_b", [D], F32, kind="ExternalInput")
    ln2_w = nc.dram_tensor("ln2_w", [D], F32, kind="ExternalInput")
    ln2_b = nc.dram_tensor("ln2_b", [D], F32, kind="ExternalInput")
    yT = nc.dram_tensor("yT", [D, NT], F32, kind="ExternalOutput")

    # pair-shared K/V exchange slabs, flat-indexed [slot*2 + half]
    slab_k = nc.dram_tensor("slab_k", [4, P, DM, 512], BF16, kind="Internal",
                            addr_space="Shared")
    slab_v = nc.dram_tensor("slab_v", [4, P, 4, H, 65], BF16, kind="Internal",
                            addr_space="Shared")
    bar_in = nc.dram_tensor("bar_in", [1, 4], F32, kind="Internal")
    bar_k = nc.dram_tensor("bar_k", [2, 4], F32, kind="Internal")
    bar_v = nc.dram_tensor("bar_v", [2, 4], F32, kind="Internal")

    tensors = dict(xT=xT, x_own=x_own, w_q=w_q, w_k=w_k, w_v=w_v, w_out=w_out, w1=w1,
                   w2=w2, b1=b1, b2=b2, ln1_w=ln1_w, ln1_b=ln1_b,
                   ln2_w=ln2_w, ln2_b=ln2_b, yT=yT, slab_k=slab_k,
                   slab_v=slab_v, bar_in=bar_in, bar_k=bar_k, bar_v=bar_v)

    with tile.TileContext(nc) as tc, \
         nc.allow_low_precision(reason="bf16 matmul operands; fp32 spine"):
        for r in range(reps):
            _emit(nc, tc, tensors, phases=phases)
    nc.compile()
    return nc


def _emit(nc, tc, T, phases=("qkv", "attn", "proj", "ffn")):
    xT_d, yT_d = T["xT"], T["yT"]
    slab_k, slab_v = T["slab_k"], T["slab_v"]

    pid = nc.partition_id()     # multi-engine: used from sync/scalar/gpsimd
    slot = pid & 1              # 0 on even cores, 1 on odd
    other = (pid + 1) & 1

    # ---------------- whole-kernel pools ----------------
    with tc.tile_pool(name="const", bufs=1) as pc, \
         tc.tile_pool(name="pers", bufs=1) as pers, \
         tc.tile_pool(name="scratch", bufs=3) as sq_pool, \
         tc.tile_pool(name="vecs", bufs=5) as vec_pool, \
         tc.tile_pool(name="psacc", bufs=2, space="PSUM") as psacc, \
         tc.tile_pool(name="pspt", bufs=2, space="PSUM") as pspt, \
         tc.tile_pool(name="psout", bufs=2, space="PSUM") as psout:

        # ---------------- constants (scalar queue) ----------------
        ones_f32 = pc.tile([P, 2 * P], F32)
        nc.vector.memset(ones_f32[:], 1.0)
        ones_col_r = ones_f32[:, 0:1].bitcast(F32R)
        ones_col = pc.tile([P, 1], BF16)          # lhsT for bf16 stat sums
        nc.vector.tensor_copy(ones_col[:], ones_f32[:, 0:1])
        ones_row = pc.tile([1, P], F32)           # lhsT for exact broadcasts
        nc.vector.tensor_copy(ones_row[:], ones_f32[0:1, 0:P])
        eps_sb = pc.tile([1, 1], F32)
        nc.vector.memset(eps_sb[:], EPS)
        b1_sb = pc.tile([P, FC], F32)
        b2_sb = pc.tile([P, DM], F32)
        lnw1_sb = pc.tile([P, DM], F32)
        lnb1_sb = pc.tile([P, DM], F32)
        lnw2_sb = pc.tile([P, DM], F32)
        lnb2_sb = pc.tile([P, DM], F32)
        bi = pc.tile([1, 4], F32)
        nc.vector.memset(bi[:], 1.0)
        dummy = pc.tile([1, 1], F32)
        nc.vector.memset(dummy[:], 1.0)

        def emit_const_dmas():
            nc.scalar.dma_start(T["bar_in"].ap(), bi[:])
            for sb, t in ((b1_sb, "b1"), (b2_sb, "b2"),
                          (lnw1_sb, "ln1_w"), (lnb1_sb, "ln1_b"),
                          (lnw2_sb, "ln2_w"), (lnb2_sb, "ln2_b")):
                nc.scalar.dma_start(
                    sb[:], T[t].ap().rearrange("(c p) -> p c", p=P))

        # persistent activations
        QT = pers.tile([P, DM, NT], BF16)
        outT = pers.tile([P, DM, NT], FP8)
        xow = pers.tile([P, DM, NT], BF16, tag="tc")
        xln18 = pers.tile([P, DM, NT], FP8)  # fp8 operand copy of xln1  # own-token x (residual 1)
        xln1 = pers.tile([P, DM, NT], BF16)     # LN1 out (ffn operand+residual)

        def ln_apply(z_tile, writes, interleave=None):
            """LayerNorm over features of z_tile [P, DM, NT] (fp32).
            writes(k, src_f32_ap) stores chunk k.  If interleave is given,
            (s1, s2) stats were already accumulated there by the caller."""
            if interleave is None:
                s1 = psacc.tile([1, NT], F32, tag="acc")
                s2 = psacc.tile([1, NT], F32, tag="acc")
                for k in range(DM):
                    eng = nc.vector if k % 2 == 0 else nc.gpsimd
                    nc.tensor.matmul(s1[:], ones_col[:], z_tile[:, k, :],
                                     start=(k == 0), stop=(k == DM - 1))
                    sq = sq_pool.tile([P, NT], BF16, tag="sq")
                    eng.tensor_mul(sq[:], z_tile[:, k, :], z_tile[:, k, :])
                    nc.tensor.matmul(s2[:], ones_col[:], sq[:],
                                     start=(k == 0), stop=(k == DM - 1))
            else:
                s1, s2 = interleave
            mu = vec_pool.tile([1, NT], F32, tag="v")
            nc.vector.tensor_scalar_mul(mu[:], s1[:], 1.0 / D)
            var = vec_pool.tile([1, NT], F32, tag="v")
            nc.vector.tensor_mul(var[:], mu[:], s1[:])
            nc.vector.tensor_sub(var[:], s2[:], var[:])
            nc.scalar.activation(var[:], var[:], AF.Sqrt, scale=1.0 / D,
                                 bias=eps_sb[:])
            rec = vec_pool.tile([1, NT], F32, tag="v")
            nc.vector.reciprocal(rec[:], var[:])
            murf = vec_pool.tile([1, NT], F32, tag="v")
            nc.vector.tensor_mul(murf[:], mu[:], rec[:])
            R = psacc.tile([P, NT], F32, tag="acc")
            nc.tensor.matmul(R[:], ones_row[:], rec[:], start=True, stop=True)
            MR = psacc.tile([P, NT], F32, tag="acc")
            nc.tensor.matmul(MR[:], ones_row[:], murf[:], start=True, stop=True)
            R_sb = vec_pool.tile([P, NT], F32, tag="v")
            nc.scalar.copy(R_sb[:], R[:])
            MR_sb = vec_pool.tile([P, NT], F32, tag="v")
            nc.scalar.copy(MR_sb[:], MR[:])
            HNT = NT // 2
            for k in range(DM):
                for h in range(2):
                    eng = nc.vector if h == 0 else nc.gpsimd
                    cols = slice(h * HNT, (h + 1) * HNT)
                    t = sq_pool.tile([P, HNT], F32, tag="sq")
                    eng.tensor_mul(t[:], z_tile[:, k, cols], R_sb[:, cols])
                    eng.tensor_sub(t[:], t[:], MR_sb[:, cols])
                    writes(k, cols, t, eng)

        with tc.tile_pool(name="ktp", bufs=1) as kt_pool, \
             tc.tile_pool(name="vpp", bufs=1) as vp_pool:

            kt = kt_pool.tile([P, DM, 1024], BF16, tag="kt")
            ktr = kt_pool.tile([P, DM, 1024], BF16, tag="ktr")
            vp = vp_pool.tile([P, 8, H * 65], BF16, tag="vp")
            vpr = vp_pool.tile([P, 8, H * 65], BF16, tag="vpr")
            vp_h = vp.rearrange("p j (h e) -> p j h e", e=65)
            vpr_h = vpr.rearrange("p j (h e) -> p j h e", e=65)
            nc.vector.tensor_copy(
                vp_h[:, :, :, 64:65],
                ones_f32[:, 0:128].rearrange("p (a b c) -> p a b c", b=H, c=1))
            if not KV_SPLIT:
                nc.vector.tensor_copy(
                    vpr_h[:, :, :, 64:65],
                    ones_f32[:, 0:128].rearrange("p (a b c) -> p a b c",
                                                 b=H, c=1))

            def kt_at(jc):
                """(tile, column-base) for rotated key chunk jc."""
                if jc < 4: return kt, jc * P
                if jc < 8: return ktr, (jc - 4) * P
                if jc < 12: return kt, (jc - 4) * P
                return ktr, (jc - 8) * P

            def vp_at(jc):
                if jc < 4: return vp, jc
                if jc < 8: return vpr, jc - 4
                if jc < 12: return vp, jc - 4
                return vpr, jc - 8

            def vp_at_h(jc):
                if jc < 4: return vp_h, jc
                if jc < 8: return vpr_h, jc - 4
                if jc < 12: return vp_h, jc - 4
                return vpr_h, jc - 8

            kw_insts, vw_insts = [], []

            with tc.tile_pool(name="xpool", bufs=1) as px:
                xT = px.tile([P, DM, NSEQ], FP8)
                xTs = _rearr(xT_d.ap())
                xTq = xTs.rearrange("p c (h q2 t) -> p c h q2 t", h=2, q2=2)
                xTt = xT.rearrange("p c (h q2 t) -> p c h q2 t", h=2, q2=2)
                # own/local quarters first so K/V of quarters {0,2} can start
                nc.sync.dma_start(xTt[:, :, 0, 0, :], xTq[:, :, 0, 0, :])
                nc.sync.dma_start(xTt[:, :, 1, 0, :], xTq[:, :, 1, 0, :])
                nc.sync.dma_start(xTt[:, :, 0, 1, :], xTq[:, :, 0, 1, :])
                nc.sync.dma_start(xTt[:, :, 1, 1, :], xTq[:, :, 1, 1, :])

                with tc.tile_pool(name="wall", bufs=1) as wall_pool:
                    wk = wall_pool.tile([P, DM, DM, P], FP8, tag="wk")
                    wks = T["w_k"].ap().rearrange("f p k t -> p f k t")
                    nc.scalar.dma_start(wk[:, 0:4], wks[:, 0:4])
                    nc.scalar.dma_start(wk[:, 4:8], wks[:, 4:8])
                    wv = wall_pool.tile([P, 2, DM, 512], FP8, tag="wv")
                    wvs = T["w_v"].ap().rearrange("f p k t -> p f k t")
                    nc.scalar.dma_start(wv[:, 0:1], wvs[:, 0:1])
                    nc.scalar.dma_start(wv[:, 1:2], wvs[:, 1:2])
                    wq = wall_pool.tile([P, DM, DM, P], FP8, tag="wq")
                    wqs = T["w_q"].ap().rearrange("f p k t -> p f k t")
                    nc.scalar.dma_start(wq[:, 0:4], wqs[:, 0:4])
                    nc.scalar.dma_start(wq[:, 4:8], wqs[:, 4:8])
                    emit_const_dmas()
                    last_xow_cp = nc.scalar.dma_start(
                        xow[:], _rearr(T["x_own"].ap()))

                    # -------- K projection --------------------------------
                    for kf in range(DM):
                        for t in ((0, 2) if KV_SPLIT else (0, 1, 2, 3)):
                            acc = psacc.tile([P, 512], F32, tag="acc")
                            for k in range(DM // 2):
                                nc.tensor.matmul(
                                    acc[:], wk[:, kf, 2 * k:2 * k + 2, :],
                                    xT[:, 2 * k:2 * k + 2,
                                       t * 512:(t + 1) * 512],
                                    start=(k == 0), stop=(k == DM // 2 - 1),
                                    perf_mode=DR)
                            dst, cb = kt_at(t * 4)
                            nc.vector.tensor_copy(
                                dst[:, kf, cb:cb + 512], acc[:])
                            if KV_SPLIT:
                                w = nc.sync.dma_start(
                                    slab_k.ap()[slot * 2 + t // 2][:, kf, :],
                                    dst[:, kf, cb:cb + 512])
                                kw_insts.append(w)

                    if KV_SPLIT and "attn" in phases and TWO_BARRIERS:
                        cck = nc.gpsimd.collective_compute(
                            "AllGather", mybir.AluOpType.bypass,
                            ins=[T["bar_in"].ap()], outs=[T["bar_k"].ap()],
                            replica_groups=[[0, 1], [2, 3], [4, 5], [6, 7]])
                        for w in kw_insts:
                            add_dep_helper(cck.ins, w.ins,
                                           reason="barrier after K writes")
                        rk1 = nc.scalar.dma_start(ktr[:, :, 0:512],
                                                slab_k.ap()[other * 2 + slot])
                        rk3 = nc.scalar.dma_start(ktr[:, :, 512:1024],
                                                slab_k.ap()[other * 2 + 1 - slot])
                        add_dep_helper(rk1.ins, cck.ins, reason="K read after barrier")
                        add_dep_helper(rk3.ins, cck.ins, reason="K read after barrier")
                        cc_last = cck

                    # -------- V projection --------------------------------
                    for dvc in range(2):  # 512 v-features = 8 heads at a time
                        for half in range(2 if KV_SPLIT else 4):
                            for jc4 in range(4):
                                jc = (half * 8 + jc4) if KV_SPLIT \
                                    else (half * 4 + jc4)
                                acc = psacc.tile([P, 512], F32, tag="acc")
                                for k in range(DM // 2):
                                    nc.tensor.matmul(
                                        acc[:],
                                        xT[:, 2 * k:2 * k + 2,
                                           jc * P:(jc + 1) * P],
                                        wv[:, dvc, 2 * k:2 * k + 2, :],
                                        start=(k == 0),
                                        stop=(k == DM // 2 - 1),
                                        perf_mode=DR)
                                dvh, jpos = vp_at_h(jc)
                                nc.vector.tensor_copy(
                                    dvh[:, jpos, dvc * 8:(dvc + 1) * 8, 0:64],
                                    acc[:].rearrange("p (h e) -> p h e", e=64))
                            if KV_SPLIT:
                                w = nc.sync.dma_start(
                                    slab_v.ap()[slot * 2 + half]
                                    [:, :, dvc * 8:(dvc + 1) * 8, :],
                                    vp_h[:, half * 4:half * 4 + 4,
                                         dvc * 8:(dvc + 1) * 8, 0:65])
                                vw_insts.append(w)

                    if KV_SPLIT and "attn" in phases:
                        ccv = nc.gpsimd.collective_compute(
                            "AllGather", mybir.AluOpType.bypass,
                            ins=[T["bar_in"].ap()], outs=[T["bar_v"].ap()],
                            replica_groups=[[0, 1], [2, 3], [4, 5], [6, 7]])
                        for w in vw_insts:
                            add_dep_helper(ccv.ins, w.ins,
                                           reason="barrier after V writes")
                        if not TWO_BARRIERS:
                            for w in kw_insts:
                                add_dep_helper(ccv.ins, w.ins,
                                               reason="barrier after K writes")
                            rk1 = nc.scalar.dma_start(ktr[:, :, 0:512],
                                                    slab_k.ap()[other * 2 + slot])
                            add_dep_helper(rk1.ins, ccv.ins,
                                           reason="K read after barrier")
                        rv1 = nc.scalar.dma_start(
                            vpr_h[:, 0:4, :, 0:65], slab_v.ap()[other * 2 + slot])
                        add_dep_helper(rv1.ins, ccv.ins, reason="V read after barrier")
                        if not TWO_BARRIERS:
                            rk3 = nc.scalar.dma_start(
                                ktr[:, :, 512:1024],
                                slab_k.ap()[other * 2 + 1 - slot])
                            add_dep_helper(rk3.ins, ccv.ins,
                                           reason="K read after barrier")
                        rv3 = nc.scalar.dma_start(
                            vpr_h[:, 4:8, :, 0:65],
                            slab_v.ap()[other * 2 + 1 - slot])
                        add_dep_helper(rv3.ins, ccv.ins, reason="V read after barrier")
                        cc_last = ccv

                    # -------- Q projection (own tokens) -------------------
                    for qf in range(DM):
                        acc = psacc.tile([P, NT], F32, tag="acc")
                        for k in range(DM // 2):
                            last_q_mm = nc.tensor.matmul(
                                acc[:], wq[:, qf, 2 * k:2 * k + 2, :],
                                xT[:, 2 * k:2 * k + 2, 0:NT],
                                start=(k == 0), stop=(k == DM // 2 - 1),
                                perf_mode=DR)
                        nc.vector.tensor_copy(QT[:, qf, :], acc[:])

            # -------- prefetch FFN weights (SWDGE; xT slot now free) -----
            _w1cm = tc.tile_pool(name="w1p", bufs=4)
            w1_pool = _w1cm.__enter__()
            _w2cm = tc.tile_pool(name="w2p", bufs=4)
            w2_pool = _w2cm.__enter__()
            w1ts, w2ts = [], []
            if "ffn" in phases:
                for fg in range(DFF // 512):
                    w1t = w1_pool.tile([P, DM, 512], FP8, tag="w1",
                                       name=f"w1t{fg}")
                    d = nc.gpsimd.dma_start(w1t[:], T["w1"].ap()[fg])
                    add_dep_helper(d.ins, last_q_mm.ins,
                                   reason="w1 slab reuses phase-1 SBUF")
                    if KV_SPLIT and "attn" in phases:
                        add_dep_helper(d.ins, cc_last.ins,
                                       reason="barrier owns Pool queue first")
                    add_dep_helper(d.ins, last_xow_cp.ins,
                                   reason="w1 slab reuses phase-1 SBUF")
                    w1ts.append(w1t)
                for ef in range(DM):
                    w2t = w2_pool.tile([P, FC, P], FP8, tag="w2",
                                       name=f"w2t{ef}")
                    d = nc.gpsimd.dma_start(w2t[:], T["w2"].ap()[ef])
                    add_dep_helper(d.ins, last_q_mm.ins,
                                   reason="w2 slab reuses phase-1 SBUF")
                    if KV_SPLIT and "attn" in phases:
                        add_dep_helper(d.ins, cc_last.ins,
                                       reason="barrier owns Pool queue first")
                    add_dep_helper(d.ins, last_xow_cp.ins,
                                   reason="w2 slab reuses phase-1 SBUF")
                    w2ts.append(w2t)

            # -------- attention (interleaved head-pair groups) ----------
            if "attn" not in phases:      # timing-bisect stub
                for k in range(DM):
                    nc.vector.tensor_copy(outT[:, k, :], QT[:, k, :])
            if "attn" in phases:
                with tc.tile_pool(name="pt", bufs=5) as pt_pool:
                    oaccs = {}
                    first_exps = []

                    def attn_chunk(hp, jcs, pool, tag, start, stop):
                        if start:
                            oaccs[hp] = [pool.tile([65, NT], F32, tag=tag,
                                                   name=f"oacc{hp}_{i}")
                                         for i in range(2)]
                        oacc = oaccs[hp]
                        for n, jc in enumerate(jcs):
                            ksrc, kcb = kt_at(jc)
                            vsrc, vpos = vp_at(jc)
                            pt_ps = pspt.tile([P, 2 * NT], F32, tag="pt")
                            for i in range(2):
                                rows = slice(64 * i, 64 * i + 64)
                                nc.tensor.matmul(
                                    pt_ps[:, i * NT:(i + 1) * NT],
                                    ksrc[rows, hp, kcb:kcb + P],
                                    QT[rows, hp, :],
                                    start=True, stop=True)
                            pt_sb = pt_pool.tile([P, 2 * NT], BF16, tag="ptsb")
                            e = nc.scalar.activation(pt_sb[:], pt_ps[:], AF.Exp,
                                                     scale=SCALE)
                            if len(first_exps) < 6:
                                first_exps.append(e)
                                add_dep_helper(e.ins, last_q_mm.ins,
                                               reason="pt_sb reuses phase-1 SBUF")
                            for i in range(2):
                                h = 2 * hp + i
                                nc.tensor.matmul(
                                    oacc[i][:],
                                    vsrc[:, vpos, h * 65:(h + 1) * 65],
                                    pt_sb[:, i * NT:(i + 1) * NT],
                                    start=(start and n == 0),
                                    stop=(stop and n == len(jcs) - 1))

                    def attn_fin(hp):
                        oacc = oaccs.pop(hp)
                        bc2 = pspt.tile([P, NT], F32, tag="pt")
                        for i in range(2):
                            rec = vec_pool.tile([1, NT], F32, tag="v")
                            nc.vector.reciprocal(rec[:], oacc[i][64:65, :])
                            nc.tensor.matmul(bc2[64 * i:64 * i + 64, :],
                                             ones_row[:, 0:64], rec[:],
                                             start=True, stop=True)
                        bc_sb = sq_pool.tile([P, NT], F32, tag="sq")
                        nc.scalar.copy(bc_sb[:], bc2[:])
                        for i in range(2):
                            nc.vector.tensor_mul(
                                outT[64 * i:64 * i + 64, hp, :],
                                oacc[i][0:64, :],
                                bc_sb[64 * i:64 * i + 64, :])

                    for hpp in range(HPAIRS // 2):
                        hpA, hpB = 2 * hpp, 2 * hpp + 1
                        attn_chunk(hpA, JC_LOCAL, psout, "o", True, False)
                        attn_chunk(hpB, JC_LOCAL, psacc, "acc", True, False)
                        attn_chunk(hpA, JC_REMOTE, psout, "o", False, True)
                        attn_fin(hpA)
                        attn_chunk(hpB, JC_REMOTE, psacc, "acc", False, True)
                        attn_fin(hpB)

            # -------- output projection + residual 1 + LN1 stats ---------
            nc.scalar.activation(dummy[:], dummy[:], AF.Sqrt)  # table preload
            z1 = kt_pool.tile([P, DM, NT], BF16, tag="kt")  # reuses kt slot
            s1 = psacc.tile([1, NT], F32, tag="acc")
            s2 = psacc.tile([1, NT], F32, tag="acc")
            with tc.tile_pool(name="wo", bufs=2) as wo_pool:
                for ef in range(DM):
                    wo = wo_pool.tile([P, DM, P], FP8)
                    nc.sync.dma_start(wo[:], T["w_out"].ap()[ef])
                    acc = pspt.tile([P, NT], F32, tag="pt")
                    for k in range(DM // 2):
                        nc.tensor.matmul(acc[:], wo[:, 2 * k:2 * k + 2, :],
                                         outT[:, 2 * k:2 * k + 2, :],
                                         start=(k == 0),
                                         stop=(k == DM // 2 - 1),
                                         perf_mode=DR)
                    nc.vector.tensor_add(z1[:, ef, :], acc[:], xow[:, ef, :])
                    nc.tensor.matmul(s1[:], ones_col[:], z1[:, ef, :],
                                     start=(ef == 0), stop=(ef == DM - 1))
                    sq = sq_pool.tile([P, NT], BF16, tag="sq")
                    nc.gpsimd.tensor_mul(sq[:], z1[:, ef, :], z1[:, ef, :])
                    nc.tensor.matmul(s2[:], ones_col[:], sq[:],
                                     start=(ef == 0), stop=(ef == DM - 1))

            # -------- LN1 ------------------------------------------------
            def write_xln1(k, cols, t, eng):
                eng.tensor_scalar(xln1[:, k, cols], t[:],
                                  lnw1_sb[:, k:k + 1],
                                  lnb1_sb[:, k:k + 1],
                                  op0=mybir.AluOpType.mult,
                                  op1=mybir.AluOpType.add)
                nc.scalar.copy(xln18[:, k, cols], xln1[:, k, cols])
            ln_apply(z1, write_xln1, interleave=(s1, s2))

            if "ffn" not in phases:   # timing-bisect stub: LN2 input
                z2 = pers.tile([P, DM, NT], BF16, tag="tc")  # xow slot
                s1 = psacc.tile([1, NT], F32, tag="acc")
                s2 = psacc.tile([1, NT], F32, tag="acc")
                for k in range(DM):
                    eng = nc.vector if k % 2 == 0 else nc.gpsimd
                    nc.vector.tensor_copy(z2[:, k, :], z1[:, k, :])
                    nc.tensor.matmul(s1[:], ones_col[:], z2[:, k, :],
                                     start=(k == 0), stop=(k == DM - 1))
                    sq = sq_pool.tile([P, NT], BF16, tag="sq")
                    eng.tensor_mul(sq[:], z2[:, k, :], z2[:, k, :])
                    nc.tensor.matmul(s2[:], ones_col[:], sq[:],
                                     start=(k == 0), stop=(k == DM - 1))

            # -------- FFN ------------------------------------------------
            if "ffn" in phases:
                hT = kt_pool.tile([P, FC, NT], FP8, tag="kt")  # kt/z1 slot
                for fg in range(DFF // 512):
                    w1t = w1ts[fg]
                    for f4 in range(4):
                        f = fg * 4 + f4
                        acc = psacc.tile([P, NT], F32, tag="acc")
                        for k in range(DM // 2):
                            nc.tensor.matmul(
                                acc[:],
                                w1t[:, 2 * k:2 * k + 2, f4 * P:(f4 + 1) * P],
                                xln18[:, 2 * k:2 * k + 2, :],
                                start=(k == 0), stop=(k == DM // 2 - 1),
                                perf_mode=DR)
                        nc.scalar.activation(hT[:, f, :], acc[:], AF.Gelu,
                                             bias=b1_sb[:, f:f + 1])

                # FFN2 with LN2 stats interleaved (stats in psacc, accs in pspt)
                nc.scalar.activation(dummy[:], dummy[:], AF.Sqrt)  # preload
                z2 = pers.tile([P, DM, NT], BF16, tag="tc")  # xow slot
                s1 = psacc.tile([1, NT], F32, tag="acc")
                s2 = psacc.tile([1, NT], F32, tag="acc")
                for ef in range(DM):
                    w2t = w2ts[ef]
                    acc = pspt.tile([P, NT], F32, tag="pt")
                    for k in range(FC // 2):
                        nc.tensor.matmul(acc[:], w2t[:, 2 * k:2 * k + 2, :],
                                         hT[:, 2 * k:2 * k + 2, :],
                                         start=(k == 0),
                                         stop=(k == FC // 2 - 1),
                                         perf_mode=DR)
                    eng = nc.vector if ef % 2 == 0 else nc.gpsimd
                    t = sq_pool.tile([P, NT], F32, tag="sq")
                    nc.scalar.activation(t[:], acc[:], AF.Identity,
                                         bias=b2_sb[:, ef:ef + 1])
                    nc.vector.tensor_add(z2[:, ef, :], t[:], xln1[:, ef, :])
                    nc.tensor.matmul(s1[:], ones_col[:], z2[:, ef, :],
                                     start=(ef == 0), stop=(ef == DM - 1))
                    sq = sq_pool.tile([P, NT], BF16, tag="sq")
                    eng.tensor_mul(sq[:], z2[:, ef, :], z2[:, ef, :])
                    nc.tensor.matmul(s2[:], ones_col[:], sq[:],
                                     start=(ef == 0), stop=(ef == DM - 1))
            _w2cm.__exit__(None, None, None)
            _w1cm.__exit__(None, None, None)

        # -------- LN2 -> output ------------------------------------------
        with tc.tile_pool(name="outstage", bufs=4) as out_pool:
            yT_r = _rearr(yT_d.ap())

            def write_out(k, cols, t, eng):
                o = out_pool.tile([P, NT // 2], F32)
                nc.scalar.activation(o[:], t[:], AF.Identity,
                                     bias=lnb2_sb[:, k:k + 1],
                                     scale=lnw2_sb[:, k:k + 1])
                q = nc.sync if k % 2 == 0 else nc.scalar
                q.dma_start(yT_r[:, k, cols], o[:])
            ln_apply(z2, write_out, interleave=(s1, s2))  # noqa: F821


def _get_nc():
    global _NC_CACHE
    if _NC_CACHE is None:
        _NC_CACHE = _build_nc()
    return _NC_CACHE


def _tile_w(W, out_cols):
    """[Din, Dout] f32 -> fp8 [Dout//out_cols, 128, Din//128, out_cols]
    so each output-chunk's weights are one contiguous DMA slab."""
    f8 = mybir.dt.np(FP8)
    Din, Dout = W.shape
    t = W.astype(f8).reshape(Din // P, P, Dout // out_cols, out_cols)
    return np.ascontiguousarray(t.transpose(2, 1, 0, 3))


def make_in_maps(x, w_qkv, w_out, ln1_w, ln1_b, w1, b1, w2, b2,
                 ln2_w, ln2_b):
    import ml_dtypes
    bf = ml_dtypes.bfloat16
    x = np.ascontiguousarray(np.asarray(x, dtype=np.float32))
    w_qkv = np.asarray(w_qkv, np.float32)
    shared = {
        "w_q": _tile_w(w_qkv[:, 0:D], P),
        "w_k": _tile_w(w_qkv[:, D:2 * D], P),
        "w_v": _tile_w(w_qkv[:, 2 * D:3 * D], 512),
        "w_out": _tile_w(np.asarray(w_out, np.float32), P),
        "w1": _tile_w(np.asarray(w1, np.float32), 512),
        "w2": _tile_w(np.asarray(w2, np.float32), P),
        "b1": np.asarray(b1, np.float32),
        "b2": np.asarray(b2, np.float32),
        "ln1_w": np.asarray(ln1_w, np.float32),
        "ln1_b": np.asarray(ln1_b, np.float32),
        "ln2_w": np.asarray(ln2_w, np.float32),
        "ln2_b": np.asarray(ln2_b, np.float32),
    }
    f8 = mybir.dt.np(FP8)
    in_maps = []
    for c in range(8):
        b, q = divmod(c, 4)
        xT = np.ascontiguousarray(x[b].T)             # [D, NSEQ]
        # rotate so this core's own tokens are always columns [0, NT)
        xTr = np.ascontiguousarray(np.roll(xT, -q * NT, axis=1))
        in_maps.append({
            "xT": np.ascontiguousarray(xTr.astype(f8)),
            "x_own": np.ascontiguousarray(xTr[:, 0:NT].astype(bf)),
            **shared,
        })
    return in_maps


def kernel(x, w_qkv, w_out, ln1_w, ln1_b, w1, b1, w2, b2, ln2_w, ln2_b):
    in_maps = make_in_maps(x, w_qkv, w_out, ln1_w, ln1_b, w1, b1, w2, b2,
                           ln2_w, ln2_b)
    nc = _get_nc()
    res = run_bass_kernel_spmd(nc, in_maps, list(range(8)))

    out = np.empty((B, NSEQ, D), np.float32)
    for c in range(8):
        b, q = divmod(c, 4)
        out[b, q * NT:(q + 1) * NT, :] = res.results[c]["yT"].T
    return out
